# revision 34
# baseline (speedup 1.0000x reference)
"""Trainium2 Bass kernel for nn_MeshTransformer (8-core SPMD, V-sharded).

Computes, for each of BS=256 (b,s) pairs:
    out[bs, v, i] = sum_{p,j} ws[bs,p] * R[i,j](bs,p) * deformed[p,v,j]
                    + sum_p w[bs,p] * t[bs,p,i]
with R the XYZ-euler rotation, ws = w * scale, deformed = base + offsets.

Mapping:
  - Vertex dim V (2562, padded to 2576) is sharded 8 ways (322/core).
  - The host precomputes every weight product in float64 and ships ready
    lhsT tiles; the device program is only DMA + PE + PSUM drains:
      out_i = LA_i^T @ DA + LB_i^T @ DB   per bs-half, where
      LA_i = [Rws_i0 (k 0..63) ; Rws_i1 (k 64..127)],  DA = [d0 ; d1]
      LB_i = [Rws_i2 (k 0..63) ; wt_i (k 64)],         DB = [d2 ; ones]
    (the ones row folds the translation term into the same contraction).
  - Inputs arrive in three chunks: c1 (LA0|DA, hot) and c2 (LA1|LA2) via
    SP/HWDGE, c3 (DB|LB) via the Pool engine's SWDGE path so its
    descriptor-gen overlaps the HWDGE chain.  PSUM groups drain on
    alternating DVE/Act (the only PSUM-capable engines) and outputs
    leave in three 2-chunk DMAs (SP, Act, SP) so the HWDGE prep of each
    overlaps the previous transfer.
  - A dummy-matmul warmup chain keeps the PE continuously busy from
    t~890ns so the pstate ramp reaches full clock before the B matmuls.
"""

import numpy as np
from contextlib import ExitStack

import concourse.bass as bass
import concourse.tile as tile
from concourse import bacc, mybir
from concourse.bass_utils import run_bass_kernel_spmd

B, S, P, V = 16, 16, 64, 2562
BS = B * S              # 256
N_CORES = 8
VPAD = 2576             # multiple of 8; per-core N kept even
VC = VPAD // N_CORES    # 322 vertices per core

F32 = mybir.dt.float32
F16 = mybir.dt.float16

# warmup chain tuning (see module docstring)
N_WARM_SMALL = 12
N_WARM_MED = 23


def _build_kernel():
    nc = bacc.Bacc("TRN2", target_bir_lowering=False, debug=False)

    c1_d = nc.dram_tensor("c1", [128, 256 + VC], F16, kind="ExternalInput").ap()
    c2_d = nc.dram_tensor("c2", [128, 512], F16, kind="ExternalInput").ap()
    c3_d = nc.dram_tensor("c3", [65, VC + 768], F16, kind="ExternalInput").ap()
    out_d = nc.dram_tensor("out", [128, 6 * VC], F16, kind="ExternalOutput").ap()

    # raw bass (no TileContext): manual semaphores avoid the Tile exit
    # barriers and per-queue teardown waits
    wsmall = nc.alloc_sbuf_tensor("wsmall", [128, 32], F16).ap()
    wmed = nc.alloc_sbuf_tensor("wmed", [128, 128], F16).ap()
    c1 = nc.alloc_sbuf_tensor("c1s", [128, 256 + VC], F16).ap()   # LA0 | DA
    c2 = nc.alloc_sbuf_tensor("c2s", [128, 512], F16).ap()        # LA1 | LA2
    c3 = nc.alloc_sbuf_tensor("c3s", [65, VC + 768], F16).ap()    # DB | LBi
    osb = nc.alloc_sbuf_tensor("osb", [128, 6 * VC], F16).ap()
    psw = nc.alloc_psum_tensor("psw", [16, 512], F32).ap()
    groups = [(0, 0), (0, 1), (1, 0), (1, 1), (2, 0), (2, 1)]
    pss = {g: nc.alloc_psum_tensor(f"ps{g[0]}{g[1]}", [128, VC], F32).ap()
           for g in groups}

    da = c1[:, 256:256 + VC]
    db = c3[:, 0:VC]

    def la(i, h):            # lhsT A-part [128, 128]
        if i == 0:
            return c1[:, h * 128:(h + 1) * 128]
        return c2[:, (i - 1) * 256 + h * 128:(i - 1) * 256 + (h + 1) * 128]

    def lb(i, h):            # lhsT B-part [65, 128]
        base = VC + i * 256 + h * 128
        return c3[:, base:base + 128]

    with ExitStack() as ctx:
        si1 = ctx.enter_context(nc.semaphore())   # c1 DMA done
        si2 = ctx.enter_context(nc.semaphore())   # c2 DMA done
        si3 = ctx.enter_context(nc.semaphore())   # c3 DMA done
        spe = ctx.enter_context(nc.semaphore())   # B-matmul completions
        sa = ctx.enter_context(nc.semaphore())    # Act drain completions
        sv = ctx.enter_context(nc.semaphore())    # DVE drain completions
        so = ctx.enter_context(nc.semaphore())    # out-DMA completions

        # Pool: c3 via SWDGE (desc-gen overlaps the HWDGE chain of c1/c2)
        nc.gpsimd.dma_start(out=c3, in_=c3_d).then_inc(si3, 16)

        # SP: the two hot input chunks, then out-DMAs 1 and 3
        nc.sync.dma_start(out=c1, in_=c1_d).then_inc(si1, 16)
        nc.sync.dma_start(out=c2, in_=c2_d).then_inc(si2, 16)

        # PE: pstate warmup chain, then the 12 real matmuls.  The warmup
        # operands are intentionally uninitialized: the results land in a
        # PSUM bank nobody reads, the PE's timing is data-independent, and
        # skipping the memset dependency lets the pstate ramp start ~170ns
        # earlier, which moves the mid->full clock boundary before B00.
        for _ in range(N_WARM_SMALL):
            nc.tensor.matmul(psw[:, 0:16], wsmall[:, 0:16], wsmall[:, 16:32],
                             start=True, stop=True)
        for _ in range(N_WARM_MED):
            nc.tensor.matmul(psw[:, 0:128], wsmall[:, 0:16], wmed[:],
                             start=True, stop=True)
        nc.tensor.wait_ge(si1, 16)
        nc.tensor.matmul(pss[(0, 0)], la(0, 0), da, start=True, stop=False)
        nc.tensor.matmul(pss[(0, 1)], la(0, 1), da, start=True, stop=False)
        nc.tensor.wait_ge(si3, 16)
        nc.tensor.matmul(pss[(0, 0)], lb(0, 0), db,
                         start=False, stop=True).then_inc(spe, 1)
        nc.tensor.matmul(pss[(0, 1)], lb(0, 1), db,
                         start=False, stop=True).then_inc(spe, 1)
        nc.tensor.wait_ge(si2, 16)
        # decode-time pacing: the PE pstate is sampled when an instruction is
        # decoded; waiting for B00's completion pushes A10's (and every
        # later matmul's) decode past the full-clock boundary without
        # delaying execution (the PE is busy until then anyway)
        nc.tensor.wait_ge(spe, 1)
        for i in (1, 2):
            nc.tensor.matmul(pss[(i, 0)], la(i, 0), da, start=True, stop=False)
            nc.tensor.matmul(pss[(i, 1)], la(i, 1), da, start=True, stop=False)
            nc.tensor.matmul(pss[(i, 0)], lb(i, 0), db,
                             start=False, stop=True).then_inc(spe, 1)
            nc.tensor.matmul(pss[(i, 1)], lb(i, 1), db,
                             start=False, stop=True).then_inc(spe, 1)

        # drains: g0/g2/g4 on Act, g1/g3/g5 on DVE (the only PSUM readers)
        for k, g in enumerate(groups):
            eng, sem = (nc.scalar, sa) if k % 2 == 0 else (nc.vector, sv)
            eng.wait_ge(spe, k + 1)
            dst = osb[:, k * VC:(k + 1) * VC]
            if k % 2 == 0:
                eng.copy(dst, pss[g]).then_inc(sem, 1)
            else:
                eng.tensor_copy(dst, pss[g]).then_inc(sem, 1)

        # out-DMAs: o1/o3 from SP, o2 from Act (its HWDGE slot chains right
        # after o1's). Completion sems are required by the NEFF lowering but
        # nothing on-chip waits on them.
        nc.sync.wait_ge(sa, 1)
        nc.sync.wait_ge(sv, 1)
        nc.sync.dma_start(out=out_d[:, 0:2 * VC],
                          in_=osb[:, 0:2 * VC]).then_inc(so, 16)
        nc.scalar.wait_ge(sa, 2)
        nc.scalar.wait_ge(sv, 2)
        nc.scalar.dma_start(out=out_d[:, 2 * VC:4 * VC],
                            in_=osb[:, 2 * VC:4 * VC]).then_inc(so, 16)
        nc.sync.wait_ge(sa, 3)
        nc.sync.wait_ge(sv, 3)
        nc.sync.dma_start(out=out_d[:, 4 * VC:6 * VC],
                          in_=osb[:, 4 * VC:6 * VC]).then_inc(so, 16)

    nc.compile()
    return nc


_NC_CACHE = None


def _get_nc():
    global _NC_CACHE
    if _NC_CACHE is None:
        _NC_CACHE = _build_kernel()
    return _NC_CACHE


def _prep_inputs(scales, transforms, prototype_weights, prototype_offsets, base_verts):
    """Host-side precompute: rotation matrices, weight folds, shard layout."""
    f = np.float64
    hh = np.float16
    scl = np.asarray(scales, f).reshape(BS, 1)
    tf = np.asarray(transforms, f).reshape(BS, P, 6)
    w = np.asarray(prototype_weights, f).reshape(BS, P)
    t = tf[:, :, 0:3]
    a, b, c = tf[:, :, 3], tf[:, :, 4], tf[:, :, 5]

    ca, sa = np.cos(a), np.sin(a)
    cb, sb = np.cos(b), np.sin(b)
    cc, sc = np.cos(c), np.sin(c)
    R = [
        [cb * cc, -cb * sc, sb],
        [ca * sc + sa * sb * cc, ca * cc - sa * sb * sc, -sa * cb],
        [sa * sc - ca * sb * cc, sa * cc + ca * sb * sc, ca * cb],
    ]
    ws = w * scl                                    # [BS, P]
    wt = np.einsum('sp,spi->is', w, t)              # [3, BS]

    # lhsT blobs (shared across cores): columns are bs, partitions are k
    LA = np.empty((128, 768), f)
    LB = np.empty((65, 768), f)
    for i in range(3):
        LA[0:64, i * 256:(i + 1) * 256] = (R[i][0] * ws).T
        LA[64:128, i * 256:(i + 1) * 256] = (R[i][1] * ws).T
        LB[0:64, i * 256:(i + 1) * 256] = (R[i][2] * ws).T
        LB[64, i * 256:(i + 1) * 256] = wt[i]
    LA = LA.astype(hh)
    LB = LB.astype(hh)

    offp = np.zeros((P, VPAD, 3), np.float32)
    offp[:, :V] = np.asarray(prototype_offsets, np.float32)
    basep = np.zeros((VPAD, 3), np.float32)
    basep[:V] = np.asarray(base_verts, np.float32)
    deformed = (basep[None] + offp).astype(hh)      # [P, VPAD, 3]

    in_maps = []
    for core in range(N_CORES):
        vs = slice(core * VC, (core + 1) * VC)
        d = deformed[:, vs, :]                      # [P, VC, 3]
        c1 = np.empty((128, 256 + VC), hh)
        c1[:, 0:256] = LA[:, 0:256]
        c1[0:64, 256:] = d[:, :, 0]
        c1[64:128, 256:] = d[:, :, 1]
        c3 = np.empty((65, VC + 768), hh)
        c3[0:64, 0:VC] = d[:, :, 2]
        c3[64, 0:VC] = 1.0
        c3[:, VC:] = LB
        in_maps.append({"c1": c1, "c2": LA[:, 256:768].copy(), "c3": c3})
    return in_maps


def kernel(scales, transforms, prototype_weights, prototype_offsets, base_verts):
    nc = _get_nc()
    in_maps = _prep_inputs(
        scales, transforms, prototype_weights, prototype_offsets, base_verts)
    res = run_bass_kernel_spmd(nc, in_maps, list(range(N_CORES)))
    full = np.empty((BS, VPAD, 3), np.float32)
    for core in range(N_CORES):
        planes = res.results[core]["out"].astype(np.float32)  # [128, 6*VC]
        vs = slice(core * VC, (core + 1) * VC)
        for g, (i, h) in enumerate([(0, 0), (0, 1), (1, 0), (1, 1), (2, 0), (2, 1)]):
            full[h * 128:(h + 1) * 128, vs, i] = planes[:, g * VC:(g + 1) * VC]
    return np.ascontiguousarray(full[:, :V, :])
